# revision 25
# baseline (speedup 1.0000x reference)
"""MoE (single shared expert) kernel for 8 trn2 NeuronCores.

Math: the reference's top-2 gating over 64 "experts" feeds a single shared
FFN, and the renormalized top-2 weights sum to s/(s+1e-9) with s >= 1/64,
i.e. 1 up to <= 6.4e-8 relative -- below f32 rounding noise.  The whole
module therefore reduces to:  out = silu(x @ up_w.T) @ down_w.T.

Sharding (8 cores): 2D = 4 token-groups x 2 expert-halves.
Each core (tg, eg) computes the partial
    ytp = ( silu(X[tg] @ up_w[eg].T) @ down_w[:, eg].T ).T      [D, TC]
with X[tg] = 2048 tokens, eg = half of the 2048 expert dims.  The host
sums the two partials of each token group and transposes back.

Schedule (from trace iteration):
* ~230-350GB/s ingest is shared round-robin over every ACTIVE transfer,
  so late tensors must not have issued descriptors early.  dn rides the
  x1 buffer ring and x2/x3 ride the x0 ring: each DMA then hardware-waits
  until its ring slot's previous tile is consumed, giving staggered,
  data-driven issue with no engine-FIFO cost at the wrong time.
* loop1 runs hybrid: di-outer blocks first (one (up[di], x[di]) pair per
  8 matmuls -- DMA pacing), eb-outer chains last so the 8 accumulation
  chains STOP staggered and the Silu ACTs pipeline on Scalar instead of
  serializing behind a joint stop.
* All DMA lines >= 1KB (512B packets halve per-queue throughput).
* y tiles: loop2(0) descriptors on Scalar (Sync is mid-stall on the dn
  ring waits), later loop2s on Sync; the final loop2 drains half-tiles
  on both engines and ends on 32KB quarters so the kernel doesn't finish
  on a 128KB write.
* ~13 warm-up matmuls hold the PE clock (HAM) at 2.4GHz through the
  ~7us engine-init + first-transfer window.
bf16 operands/outputs (err ~4e-3 vs the 2e-2 gate), fused Silu.
"""

import os
import sys

import numpy as np

for _p in ("/opt/trn_rl_repo",):
    if os.path.isdir(_p) and _p not in sys.path:
        sys.path.insert(0, _p)

import concourse.bass as bass
import concourse.mybir as mybir
import concourse.tile as tile

F32 = mybir.dt.float32
F32R = mybir.dt.float32r
BF16 = mybir.dt.bfloat16


def _ensure_axon_hooks_shim():
    """bass_utils' trace path imports antenv.axon_hooks, which this image
    lacks; give it a no-op hook module so BASS_TRACE=1 degrades gracefully."""
    import types
    if "antenv.axon_hooks" in sys.modules:
        return
    try:
        import antenv
    except ImportError:
        return
    if hasattr(antenv, "axon_hooks"):
        return
    ah = types.ModuleType("antenv.axon_hooks")
    ah._hook = None
    ah.set_axon_ntff_profile_hook = lambda h: setattr(ah, "_hook", h)
    ah.get_axon_ntff_profile_hook = lambda: ah._hook
    sys.modules["antenv.axon_hooks"] = ah
    antenv.axon_hooks = ah


_ensure_axon_hooks_shim()


def _split_multi_waits(nc):
    """This container's walrus encodes at most ONE sync wait per engine
    instruction ("Too many sync wait commands").  Tile routinely emits
    instructions waiting on several semaphores; hoist the extra waits onto
    single-wait NoOps inserted just before, on the same engine."""
    n = 0
    for f in nc.m.functions:
        for blk in f.blocks:
            insts = blk.instructions
            out = []
            for inst in insts:
                si = inst.sync_info
                waits = list(si.on_wait) if si and si.on_wait else []
                if len(waits) > 1:
                    for w in waits[:-1]:
                        n += 1
                        nop = mybir.InstNoOp(name=f"I-wsplit-{n}", ins=[], outs=[])
                        nop.engine = inst.engine
                        nop.sync_info = mybir.SyncInfo(on_wait=[w], on_update=[])
                        nc.register_instruction(nop)
                        out.append(nop)
                    si.on_wait = [waits[-1]]
                out.append(inst)
            if n:
                insts[:] = out
    return n

# Problem shape (hardcoded per contract)
B, S, D, ED = 4, 2048, 1024, 2048
T = B * S                    # 8192 tokens
TG, EG = 4, 2                # token groups x expert-half groups = 8 cores
TC = T // TG                 # tokens per core      = 2048
EC = ED // EG                # expert dims per core = 1024
TT = 512                     # token tile (matmul free dim)
NTT = TC // TT               # 4 token tiles
NDT = D // 128               # 8 d-tiles (contraction 1 / output rows)
NET = EC // 128              # 8 e-tiles (output rows 1 / contraction 2)
HT = TT // 2                 # half token tile

_CACHE = {}
LAST_RESULTS = None          # BassKernelResults of the most recent run


def build_nc(mode: str = "bf16") -> bass.Bass:
    """One-core SPMD program: ytp[D, TC] = (silu(x @ upT) @ dwnT).T partial."""
    mm_dt = {"bf16": BF16, "f32r": F32R, "f32": F32}[mode]
    out_dt = BF16 if mode == "bf16" else F32

    nc = bass.Bass()
    xt = nc.dram_tensor("xt", [D, TC], mm_dt, kind="ExternalInput")
    upw = nc.dram_tensor("upw", [D, EC], mm_dt, kind="ExternalInput")
    dwn = nc.dram_tensor("dwn", [EC, D], mm_dt, kind="ExternalInput")
    ytp = nc.dram_tensor("ytp", [D, TC], out_dt, kind="ExternalOutput")
    # Tiny scratch output whose DMA gates Sync's late-descriptor stream on
    # h(tt0, eb0) being ready (~27us) -- a data-driven FIFO stall.
    hgate = nc.dram_tensor("hgate", [128, 16], mm_dt, kind="ExternalOutput")

    n_warm = int(os.environ.get("MOE_WARM_MM", "13"))
    fuse_silu = os.environ.get("MOE_FUSE_SILU", "1") == "1"

    with tile.TileContext(nc) as tc:
        with (
            tc.tile_pool(name="wpool", bufs=1) as wpool,
            tc.tile_pool(name="xpool", bufs=8) as xpool,
            tc.tile_pool(name="hpool", bufs=20) as hpool,
            tc.tile_pool(name="ypool", bufs=8) as ypool,
            tc.tile_pool(name="psum", bufs=8, space="PSUM") as psum,
        ):
            # up[0] as two [128, 512] halves (first-matmul gate is one
            # 128KB transfer); up[1..7] whole rows (2KB DMA lines).
            up0_sb = [wpool.tile([128, EC // 2], mm_dt, tag=f"up0_{c}",
                                 name=f"up0_{c}") for c in range(2)]
            upr_sb = [wpool.tile([128, EC], mm_dt, tag=f"up{di}",
                                 name=f"up{di}") for di in range(1, NDT)]

            def up_slice(di, eb):
                if di == 0:
                    c, r = divmod(eb, 4)
                    return up0_sb[c][:, r * 128:(r + 1) * 128]
                return upr_sb[di - 1][:, eb * 128:(eb + 1) * 128]

            # --- PE pre-warm through the engine-init window --------------
            if n_warm:
                wz = wpool.tile([128, 128], mm_dt, tag="warmw")
                xz = wpool.tile([128, TT], mm_dt, tag="warmx")
                nc.vector.memset(wz[:], 0.0)
                nc.vector.memset(xz[:], 0.0)
                wps = psum.tile([128, TT], F32, tag="ps", name="warm_ps")
                for _ in range(n_warm):
                    nc.tensor.matmul(wps[:], wz[:], xz[:], start=True, stop=True)
                wsink = wpool.tile([128, TT], F32, tag="warmy")
                nc.vector.tensor_copy(wsink[:], wps[:])

            # --- DMA descriptor streams (strict consumption order) -------
            # Sync: up0a, x0[0], up0b, up[1..7], then the ring-gated late
            # tensors (dn halves, x2 halves, x3 halves -- each stalls on
            # its ring slot, and Sync has nothing else to do until then).
            # Scalar: x0[1..7], x1[0..7], then the Silu ACTs as emitted.
            nc.sync.dma_start(out=up0_sb[0][:], in_=upw[0:128, 0:512])
            x0_sb = []
            t = xpool.tile([128, TT], mm_dt, tag="x0", name="x0_0", bufs=8)
            nc.sync.dma_start(out=t[:], in_=xt[0:128, 0:TT])
            x0_sb.append(t)
            nc.sync.dma_start(out=up0_sb[1][:], in_=upw[0:128, 512:1024])
            for di in range(1, 6):
                nc.sync.dma_start(out=upr_sb[di - 1][:],
                                  in_=upw[di * 128:(di + 1) * 128, :])
            # up[6], up[7] on Scalar's first slots: as sync descriptors #9/#10
            # their transfers tail-ended the up stream and stalled loop1(0)'s
            # eb-chains ~1us (measured 23.5-24.5us gap).
            for di in range(6, NDT):
                nc.scalar.dma_start(out=upr_sb[di - 1][:],
                                    in_=upw[di * 128:(di + 1) * 128, :])

            for di in range(1, NDT):
                t = xpool.tile([128, TT], mm_dt, tag="x0", name=f"x0_{di}",
                               bufs=8)
                nc.scalar.dma_start(out=t[:], in_=xt[di * 128:(di + 1) * 128, 0:TT])
                x0_sb.append(t)
            x1_sb = []
            for di in range(NDT):
                t = xpool.tile([128, TT], mm_dt, tag="x1", name=f"x1_{di}",
                               bufs=8)
                nc.scalar.dma_start(out=t[:],
                                    in_=xt[di * 128:(di + 1) * 128, TT:2 * TT])
                x1_sb.append(t)

            # dn is live to the end of the kernel: persistent wpool tiles,
            # with descriptors emitted later behind the hgate stall (below).
            dn_sb = [wpool.tile([128, D], mm_dt, tag=f"dn{ei}",
                                name=f"dn{ei}") for ei in range(NET)]

            def dn_slice(ei, db):
                return dn_sb[ei][:, db * 128:(db + 1) * 128]

            xs_all = {0: x0_sb, 1: x1_sb}
            hs_all = {}

            def late_descs():
                # Emitted after loop1(0): Sync's FIFO first stalls on the
                # hgate DMA (reads h(tt0,0), ready ~27us), so none of these
                # transfers steal round-robin ingest from up/x0/x1.
                nc.sync.dma_start(out=hgate[:, :], in_=hs_all[0][2][:, 0:16])
                for ei in range(NET):
                    nc.sync.dma_start(out=dn_sb[ei][:],
                                      in_=dwn[ei * 128:(ei + 1) * 128, :])
                # x2 rides the x0 ring (slot di freed by loop1(0) block di);
                # x3 rides the same ring one lap later (freed as loop1(2)
                # consumes x2[di]).
                x2_sb, x3_sb = [], []
                for di in range(NDT):
                    t = xpool.tile([128, TT], mm_dt, tag="x0",
                                   name=f"x2_{di}", bufs=8)
                    nc.sync.dma_start(
                        out=t[:], in_=xt[di * 128:(di + 1) * 128, 2 * TT:3 * TT])
                    x2_sb.append(t)
                for di in range(NDT):
                    t = xpool.tile([128, TT], mm_dt, tag="x0",
                                   name=f"x3_{di}", bufs=8)
                    nc.sync.dma_start(
                        out=t[:], in_=xt[di * 128:(di + 1) * 128, 3 * TT:4 * TT])
                    x3_sb.append(t)
                xs_all[2] = x2_sb
                xs_all[3] = x3_sb

            def silu(ps):
                """h = silu(ps) in mm_dt, ready as a loop2 moving operand."""
                h = hpool.tile([128, TT], mm_dt, tag="h", bufs=20)
                if fuse_silu:
                    nc.scalar.activation(
                        h[:], ps[:], mybir.ActivationFunctionType.Silu
                    )
                else:
                    sg = hpool.tile([128, TT], F32, tag="sg", bufs=3)
                    nc.scalar.activation(
                        sg[:], ps[:], mybir.ActivationFunctionType.Sigmoid
                    )
                    nc.vector.tensor_mul(h[:], ps[:], sg[:])
                return h

            def loop1(tt, split):
                # Hybrid: di-outer for di < split, then eb-outer chains.
                xs = xs_all[tt]
                pss = [psum.tile([128, TT], F32, tag="ps",
                                 name=f"ps1_{tt}_{eb}")
                       for eb in range(NET)]
                for di in range(split):
                    for eb in range(NET):
                        nc.tensor.matmul(
                            pss[eb][:], up_slice(di, eb), xs[di][:],
                            start=(di == 0), stop=False,
                        )
                hs = []
                for eb in range(NET):
                    for di in range(split, NDT):
                        nc.tensor.matmul(
                            pss[eb][:], up_slice(di, eb), xs[di][:],
                            start=(di == 0), stop=(di == NDT - 1),
                        )
                    hs.append(silu(pss[eb]))
                hs_all[tt] = hs

            def emit_y(ps, tt, db, eng):
                t0 = tt * TT
                y = ypool.tile([128, TT], out_dt, tag="y")
                nc.vector.tensor_copy(y[:], ps[:])
                eng.dma_start(out=ytp[db * 128:(db + 1) * 128, t0:t0 + TT],
                              in_=y[:])

            def loop2(tt, ei_major, yeng):
                t0 = tt * TT
                hs = hs_all.pop(tt)
                if ei_major:
                    ps2s = [psum.tile([128, TT], F32, tag="ps",
                                      name=f"ps2_{tt}_{db}")
                            for db in range(NDT)]
                    for ei in range(NET):
                        for db in range(NDT):
                            nc.tensor.matmul(
                                ps2s[db][:], dn_slice(ei, db), hs[ei][:],
                                start=(ei == 0), stop=(ei == NET - 1),
                            )
                    for db in range(NDT):
                        emit_y(ps2s[db], tt, db, yeng)
                    return
                ycnt = 0
                for db in range(NDT):
                    if tt == NTT - 1 and db == NDT - 1:
                        # Final tile: two half-token chains, one per engine,
                        # so the kernel ends on a ~64KB transfer.
                        dsl = slice(db * 128, (db + 1) * 128)
                        for h0, eng in ((0, nc.sync), (HT, nc.scalar)):
                            psh = psum.tile([128, HT], F32, tag="ps",
                                            name=f"ps2h_{db}_{h0}")
                            for ei in range(NET):
                                nc.tensor.matmul(
                                    psh[:], dn_slice(ei, db),
                                    hs[ei][:, h0:h0 + HT],
                                    start=(ei == 0), stop=(ei == NET - 1),
                                )
                            yh = ypool.tile([128, HT], out_dt,
                                            tag="y2", bufs=4)
                            nc.vector.tensor_copy(yh[:], psh[:])
                            eng.dma_start(
                                out=ytp[dsl, t0 + h0:t0 + h0 + HT],
                                in_=yh[:])
                        continue
                    ps2 = psum.tile([128, TT], F32, tag="ps",
                                    name=f"ps2_{tt}_{db}")
                    for ei in range(NET):
                        nc.tensor.matmul(
                            ps2[:], dn_slice(ei, db), hs[ei][:],
                            start=(ei == 0), stop=(ei == NET - 1),
                        )
                    if tt == NTT - 1:
                        # Whole [128,512] tiles (1KB write lines -- halving
                        # them to 512B lines halved write throughput right
                        # at the peak-demand drain), engines alternated.
                        eng = nc.sync if ycnt % 2 == 0 else nc.scalar
                        ycnt += 1
                        emit_y(ps2, tt, db, eng)
                    else:
                        emit_y(ps2, tt, db, yeng)

            loop1(0, split=5)
            late_descs()
            loop1(1, split=4)
            loop2(0, ei_major=True, yeng=nc.scalar)
            loop1(2, split=3)
            loop2(1, ei_major=False, yeng=nc.sync)
            loop1(3, split=3)
            loop2(2, ei_major=False, yeng=nc.sync)
            loop2(3, ei_major=False, yeng=None)

    _split_multi_waits(nc)
    nc.finalize()
    return nc


def _get_nc(mode: str) -> bass.Bass:
    if mode not in _CACHE:
        _CACHE[mode] = build_nc(mode)
    return _CACHE[mode]


def kernel(x, gate_w, up_w, down_w):
    global LAST_RESULTS
    from concourse.bass_utils import run_bass_kernel_spmd

    mode = os.environ.get("MOE_MM_DTYPE", "bf16")
    nc = _get_nc(mode)

    if mode == "bf16":
        import ml_dtypes
        host_dt = ml_dtypes.bfloat16
    else:
        host_dt = np.float32

    xf = np.asarray(x, dtype=np.float32).reshape(T, D)
    up = np.asarray(up_w, dtype=np.float32)
    dn = np.asarray(down_w, dtype=np.float32)

    xts = [np.ascontiguousarray(xf[tg * TC:(tg + 1) * TC, :].T).astype(host_dt)
           for tg in range(TG)]
    upts = [np.ascontiguousarray(up[eg * EC:(eg + 1) * EC, :].T).astype(host_dt)
            for eg in range(EG)]
    dnts = [np.ascontiguousarray(dn[:, eg * EC:(eg + 1) * EC].T).astype(host_dt)
            for eg in range(EG)]

    in_maps = []
    for c in range(8):
        tg, eg = c // EG, c % EG
        in_maps.append({"xt": xts[tg], "upw": upts[eg], "dwn": dnts[eg]})

    res = run_bass_kernel_spmd(nc, in_maps, list(range(8)))
    LAST_RESULTS = res

    out = np.empty((T, D), dtype=np.float32)
    for tg in range(TG):
        part = (res.results[tg * EG]["ytp"].astype(np.float32)
                + res.results[tg * EG + 1]["ytp"].astype(np.float32))
        out[tg * TC:(tg + 1) * TC, :] = part.T
    return out.reshape(B, S, D)


# revision 26
# speedup vs baseline: 1.0007x; 1.0007x over previous
"""MoE (single shared expert) kernel for 8 trn2 NeuronCores.

Math: the reference's top-2 gating over 64 "experts" feeds a single shared
FFN, and the renormalized top-2 weights sum to s/(s+1e-9) with s >= 1/64,
i.e. 1 up to <= 6.4e-8 relative -- below f32 rounding noise.  The whole
module therefore reduces to:  out = silu(x @ up_w.T) @ down_w.T.

Sharding (8 cores): 2D = 4 token-groups x 2 expert-halves.
Each core (tg, eg) computes the partial
    ytp = ( silu(X[tg] @ up_w[eg].T) @ down_w[:, eg].T ).T      [D, TC]
with X[tg] = 2048 tokens, eg = half of the 2048 expert dims.  The host
sums the two partials of each token group and transposes back.

Schedule (from trace iteration):
* ~230-350GB/s ingest is shared round-robin over every ACTIVE transfer,
  so late tensors must not have issued descriptors early.  dn rides the
  x1 buffer ring and x2/x3 ride the x0 ring: each DMA then hardware-waits
  until its ring slot's previous tile is consumed, giving staggered,
  data-driven issue with no engine-FIFO cost at the wrong time.
* loop1 runs hybrid: di-outer blocks first (one (up[di], x[di]) pair per
  8 matmuls -- DMA pacing), eb-outer chains last so the 8 accumulation
  chains STOP staggered and the Silu ACTs pipeline on Scalar instead of
  serializing behind a joint stop.
* All DMA lines >= 1KB (512B packets halve per-queue throughput).
* y tiles: loop2(0) descriptors on Scalar (Sync is mid-stall on the dn
  ring waits), later loop2s on Sync; the final loop2 drains half-tiles
  on both engines and ends on 32KB quarters so the kernel doesn't finish
  on a 128KB write.
* ~13 warm-up matmuls hold the PE clock (HAM) at 2.4GHz through the
  ~7us engine-init + first-transfer window.
bf16 operands/outputs (err ~4e-3 vs the 2e-2 gate), fused Silu.
"""

import os
import sys

import numpy as np

for _p in ("/opt/trn_rl_repo",):
    if os.path.isdir(_p) and _p not in sys.path:
        sys.path.insert(0, _p)

import concourse.bass as bass
import concourse.mybir as mybir
import concourse.tile as tile

F32 = mybir.dt.float32
F32R = mybir.dt.float32r
BF16 = mybir.dt.bfloat16


def _ensure_axon_hooks_shim():
    """bass_utils' trace path imports antenv.axon_hooks, which this image
    lacks; give it a no-op hook module so BASS_TRACE=1 degrades gracefully."""
    import types
    if "antenv.axon_hooks" in sys.modules:
        return
    try:
        import antenv
    except ImportError:
        return
    if hasattr(antenv, "axon_hooks"):
        return
    ah = types.ModuleType("antenv.axon_hooks")
    ah._hook = None
    ah.set_axon_ntff_profile_hook = lambda h: setattr(ah, "_hook", h)
    ah.get_axon_ntff_profile_hook = lambda: ah._hook
    sys.modules["antenv.axon_hooks"] = ah
    antenv.axon_hooks = ah


_ensure_axon_hooks_shim()


def _split_multi_waits(nc):
    """This container's walrus encodes at most ONE sync wait per engine
    instruction ("Too many sync wait commands").  Tile routinely emits
    instructions waiting on several semaphores; hoist the extra waits onto
    single-wait NoOps inserted just before, on the same engine."""
    n = 0
    for f in nc.m.functions:
        for blk in f.blocks:
            insts = blk.instructions
            out = []
            for inst in insts:
                si = inst.sync_info
                waits = list(si.on_wait) if si and si.on_wait else []
                if len(waits) > 1:
                    for w in waits[:-1]:
                        n += 1
                        nop = mybir.InstNoOp(name=f"I-wsplit-{n}", ins=[], outs=[])
                        nop.engine = inst.engine
                        nop.sync_info = mybir.SyncInfo(on_wait=[w], on_update=[])
                        nc.register_instruction(nop)
                        out.append(nop)
                    si.on_wait = [waits[-1]]
                out.append(inst)
            if n:
                insts[:] = out
    return n

# Problem shape (hardcoded per contract)
B, S, D, ED = 4, 2048, 1024, 2048
T = B * S                    # 8192 tokens
TG, EG = 4, 2                # token groups x expert-half groups = 8 cores
TC = T // TG                 # tokens per core      = 2048
EC = ED // EG                # expert dims per core = 1024
TT = 512                     # token tile (matmul free dim)
NTT = TC // TT               # 4 token tiles
NDT = D // 128               # 8 d-tiles (contraction 1 / output rows)
NET = EC // 128              # 8 e-tiles (output rows 1 / contraction 2)
HT = TT // 2                 # half token tile

_CACHE = {}
LAST_RESULTS = None          # BassKernelResults of the most recent run


def build_nc(mode: str = "bf16") -> bass.Bass:
    """One-core SPMD program: ytp[D, TC] = (silu(x @ upT) @ dwnT).T partial."""
    mm_dt = {"bf16": BF16, "f32r": F32R, "f32": F32}[mode]
    out_dt = BF16 if mode == "bf16" else F32

    nc = bass.Bass()
    xt = nc.dram_tensor("xt", [D, TC], mm_dt, kind="ExternalInput")
    upw = nc.dram_tensor("upw", [D, EC], mm_dt, kind="ExternalInput")
    dwn = nc.dram_tensor("dwn", [EC, D], mm_dt, kind="ExternalInput")
    ytp = nc.dram_tensor("ytp", [D, TC], out_dt, kind="ExternalOutput")
    # Tiny scratch output whose DMA gates Sync's late-descriptor stream on
    # h(tt0, eb0) being ready (~27us) -- a data-driven FIFO stall.
    hgate = nc.dram_tensor("hgate", [128, 16], mm_dt, kind="ExternalOutput")

    n_warm = int(os.environ.get("MOE_WARM_MM", "13"))
    fuse_silu = os.environ.get("MOE_FUSE_SILU", "1") == "1"

    with tile.TileContext(nc) as tc:
        with (
            tc.tile_pool(name="wpool", bufs=1) as wpool,
            tc.tile_pool(name="xpool", bufs=8) as xpool,
            tc.tile_pool(name="hpool", bufs=20) as hpool,
            tc.tile_pool(name="ypool", bufs=8) as ypool,
            tc.tile_pool(name="psum", bufs=8, space="PSUM") as psum,
        ):
            # up[0] as two [128, 512] halves (first-matmul gate is one
            # 128KB transfer); up[1..7] whole rows (2KB DMA lines).
            up0_sb = [wpool.tile([128, EC // 2], mm_dt, tag=f"up0_{c}",
                                 name=f"up0_{c}") for c in range(2)]
            upr_sb = [wpool.tile([128, EC], mm_dt, tag=f"up{di}",
                                 name=f"up{di}") for di in range(1, NDT)]

            def up_slice(di, eb):
                if di == 0:
                    c, r = divmod(eb, 4)
                    return up0_sb[c][:, r * 128:(r + 1) * 128]
                return upr_sb[di - 1][:, eb * 128:(eb + 1) * 128]

            # --- PE pre-warm through the engine-init window --------------
            if n_warm:
                wz = wpool.tile([128, 128], mm_dt, tag="warmw")
                xz = wpool.tile([128, TT], mm_dt, tag="warmx")
                nc.vector.memset(wz[:], 0.0)
                nc.vector.memset(xz[:], 0.0)
                wps = psum.tile([128, TT], F32, tag="ps", name="warm_ps")
                for _ in range(n_warm):
                    nc.tensor.matmul(wps[:], wz[:], xz[:], start=True, stop=True)
                wsink = wpool.tile([128, TT], F32, tag="warmy")
                nc.vector.tensor_copy(wsink[:], wps[:])

            # --- DMA descriptor streams (strict consumption order) -------
            # Sync: up0a, x0[0], up0b, up[1..7], then the ring-gated late
            # tensors (dn halves, x2 halves, x3 halves -- each stalls on
            # its ring slot, and Sync has nothing else to do until then).
            # Scalar: x0[1..7], x1[0..7], then the Silu ACTs as emitted.
            nc.sync.dma_start(out=up0_sb[0][:], in_=upw[0:128, 0:512])
            x0_sb = []
            t = xpool.tile([128, TT], mm_dt, tag="x0", name="x0_0", bufs=8)
            nc.sync.dma_start(out=t[:], in_=xt[0:128, 0:TT])
            x0_sb.append(t)
            nc.sync.dma_start(out=up0_sb[1][:], in_=upw[0:128, 512:1024])
            for di in range(1, 6):
                nc.sync.dma_start(out=upr_sb[di - 1][:],
                                  in_=upw[di * 128:(di + 1) * 128, :])
            # up[6], up[7] on Scalar's first slots: as sync descriptors #9/#10
            # their transfers tail-ended the up stream and stalled loop1(0)'s
            # eb-chains ~1us (measured 23.5-24.5us gap).
            for di in range(6, NDT):
                nc.scalar.dma_start(out=upr_sb[di - 1][:],
                                    in_=upw[di * 128:(di + 1) * 128, :])

            for di in range(1, NDT):
                t = xpool.tile([128, TT], mm_dt, tag="x0", name=f"x0_{di}",
                               bufs=8)
                nc.scalar.dma_start(out=t[:], in_=xt[di * 128:(di + 1) * 128, 0:TT])
                x0_sb.append(t)
            x1_sb = []
            for di in range(NDT):
                t = xpool.tile([128, TT], mm_dt, tag="x1", name=f"x1_{di}",
                               bufs=8)
                nc.scalar.dma_start(out=t[:],
                                    in_=xt[di * 128:(di + 1) * 128, TT:2 * TT])
                x1_sb.append(t)

            # dn is live to the end of the kernel: persistent wpool tiles,
            # with descriptors emitted later behind the hgate stall (below).
            dn_sb = [wpool.tile([128, D], mm_dt, tag=f"dn{ei}",
                                name=f"dn{ei}") for ei in range(NET)]

            def dn_slice(ei, db):
                return dn_sb[ei][:, db * 128:(db + 1) * 128]

            xs_all = {0: x0_sb, 1: x1_sb}
            hs_all = {}

            def late_descs():
                # Emitted after loop1(0): Sync's FIFO first stalls on the
                # hgate DMA (reads h(tt0,0), ready ~27us), so none of these
                # transfers steal round-robin ingest from up/x0/x1.
                nc.sync.dma_start(out=hgate[:, :], in_=hs_all[0][2][:, 0:16])
                for ei in range(NET):
                    nc.sync.dma_start(out=dn_sb[ei][:],
                                      in_=dwn[ei * 128:(ei + 1) * 128, :])
                # x2 rides the x0 ring (slot di freed by loop1(0) block di);
                # x3 rides the same ring one lap later (freed as loop1(2)
                # consumes x2[di]).
                x2_sb, x3_sb = [], []
                for di in range(NDT):
                    t = xpool.tile([128, TT], mm_dt, tag="x0",
                                   name=f"x2_{di}", bufs=8)
                    nc.sync.dma_start(
                        out=t[:], in_=xt[di * 128:(di + 1) * 128, 2 * TT:3 * TT])
                    x2_sb.append(t)
                for di in range(NDT):
                    t = xpool.tile([128, TT], mm_dt, tag="x0",
                                   name=f"x3_{di}", bufs=8)
                    nc.sync.dma_start(
                        out=t[:], in_=xt[di * 128:(di + 1) * 128, 3 * TT:4 * TT])
                    x3_sb.append(t)
                xs_all[2] = x2_sb
                xs_all[3] = x3_sb

            def silu(ps):
                """h = silu(ps) in mm_dt, ready as a loop2 moving operand.

                The idle DVE copies the pre-activation out of PSUM (casting
                to bf16) so the bank frees at copy speed (~430ns) instead of
                waiting on the 690ns Silu ACTs, whose Scalar-engine pipeline
                otherwise lags the staggered chain stops; the ACT then runs
                off the critical path from SBUF."""
                h = hpool.tile([128, TT], mm_dt, tag="h", bufs=20)
                if fuse_silu:
                    zt = hpool.tile([128, TT], mm_dt, tag="z", bufs=10)
                    nc.vector.tensor_copy(zt[:], ps[:])
                    nc.scalar.activation(
                        h[:], zt[:], mybir.ActivationFunctionType.Silu
                    )
                else:
                    sg = hpool.tile([128, TT], F32, tag="sg", bufs=3)
                    nc.scalar.activation(
                        sg[:], ps[:], mybir.ActivationFunctionType.Sigmoid
                    )
                    nc.vector.tensor_mul(h[:], ps[:], sg[:])
                return h

            def loop1(tt, split):
                # Hybrid: di-outer for di < split, then eb-outer chains.
                xs = xs_all[tt]
                pss = [psum.tile([128, TT], F32, tag="ps",
                                 name=f"ps1_{tt}_{eb}")
                       for eb in range(NET)]
                for di in range(split):
                    for eb in range(NET):
                        nc.tensor.matmul(
                            pss[eb][:], up_slice(di, eb), xs[di][:],
                            start=(di == 0), stop=False,
                        )
                hs = []
                for eb in range(NET):
                    for di in range(split, NDT):
                        nc.tensor.matmul(
                            pss[eb][:], up_slice(di, eb), xs[di][:],
                            start=(di == 0), stop=(di == NDT - 1),
                        )
                    hs.append(silu(pss[eb]))
                hs_all[tt] = hs

            def emit_y(ps, tt, db, eng):
                t0 = tt * TT
                y = ypool.tile([128, TT], out_dt, tag="y")
                nc.vector.tensor_copy(y[:], ps[:])
                eng.dma_start(out=ytp[db * 128:(db + 1) * 128, t0:t0 + TT],
                              in_=y[:])

            def loop2(tt, ei_major, yeng):
                t0 = tt * TT
                hs = hs_all.pop(tt)
                if ei_major:
                    ps2s = [psum.tile([128, TT], F32, tag="ps",
                                      name=f"ps2_{tt}_{db}")
                            for db in range(NDT)]
                    for ei in range(NET):
                        for db in range(NDT):
                            nc.tensor.matmul(
                                ps2s[db][:], dn_slice(ei, db), hs[ei][:],
                                start=(ei == 0), stop=(ei == NET - 1),
                            )
                    for db in range(NDT):
                        emit_y(ps2s[db], tt, db, yeng)
                    return
                ycnt = 0
                for db in range(NDT):
                    if tt == NTT - 1 and db == NDT - 1:
                        # Final tile: two half-token chains, one per engine,
                        # so the kernel ends on a ~64KB transfer.
                        dsl = slice(db * 128, (db + 1) * 128)
                        for h0, eng in ((0, nc.sync), (HT, nc.scalar)):
                            psh = psum.tile([128, HT], F32, tag="ps",
                                            name=f"ps2h_{db}_{h0}")
                            for ei in range(NET):
                                nc.tensor.matmul(
                                    psh[:], dn_slice(ei, db),
                                    hs[ei][:, h0:h0 + HT],
                                    start=(ei == 0), stop=(ei == NET - 1),
                                )
                            yh = ypool.tile([128, HT], out_dt,
                                            tag="y2", bufs=4)
                            nc.vector.tensor_copy(yh[:], psh[:])
                            eng.dma_start(
                                out=ytp[dsl, t0 + h0:t0 + h0 + HT],
                                in_=yh[:])
                        continue
                    ps2 = psum.tile([128, TT], F32, tag="ps",
                                    name=f"ps2_{tt}_{db}")
                    for ei in range(NET):
                        nc.tensor.matmul(
                            ps2[:], dn_slice(ei, db), hs[ei][:],
                            start=(ei == 0), stop=(ei == NET - 1),
                        )
                    if tt == NTT - 1:
                        # Whole [128,512] tiles (1KB write lines -- halving
                        # them to 512B lines halved write throughput right
                        # at the peak-demand drain), engines alternated.
                        eng = nc.sync if ycnt % 2 == 0 else nc.scalar
                        ycnt += 1
                        emit_y(ps2, tt, db, eng)
                    else:
                        emit_y(ps2, tt, db, yeng)

            loop1(0, split=5)
            late_descs()
            loop1(1, split=4)
            loop2(0, ei_major=True, yeng=nc.scalar)
            loop1(2, split=3)
            loop2(1, ei_major=False, yeng=nc.sync)
            loop1(3, split=3)
            loop2(2, ei_major=False, yeng=nc.sync)
            loop2(3, ei_major=False, yeng=None)

    _split_multi_waits(nc)
    nc.finalize()
    return nc


def _get_nc(mode: str) -> bass.Bass:
    if mode not in _CACHE:
        _CACHE[mode] = build_nc(mode)
    return _CACHE[mode]


def kernel(x, gate_w, up_w, down_w):
    global LAST_RESULTS
    from concourse.bass_utils import run_bass_kernel_spmd

    mode = os.environ.get("MOE_MM_DTYPE", "bf16")
    nc = _get_nc(mode)

    if mode == "bf16":
        import ml_dtypes
        host_dt = ml_dtypes.bfloat16
    else:
        host_dt = np.float32

    xf = np.asarray(x, dtype=np.float32).reshape(T, D)
    up = np.asarray(up_w, dtype=np.float32)
    dn = np.asarray(down_w, dtype=np.float32)

    xts = [np.ascontiguousarray(xf[tg * TC:(tg + 1) * TC, :].T).astype(host_dt)
           for tg in range(TG)]
    upts = [np.ascontiguousarray(up[eg * EC:(eg + 1) * EC, :].T).astype(host_dt)
            for eg in range(EG)]
    dnts = [np.ascontiguousarray(dn[:, eg * EC:(eg + 1) * EC].T).astype(host_dt)
            for eg in range(EG)]

    in_maps = []
    for c in range(8):
        tg, eg = c // EG, c % EG
        in_maps.append({"xt": xts[tg], "upw": upts[eg], "dwn": dnts[eg]})

    res = run_bass_kernel_spmd(nc, in_maps, list(range(8)))
    LAST_RESULTS = res

    out = np.empty((T, D), dtype=np.float32)
    for tg in range(TG):
        part = (res.results[tg * EG]["ytp"].astype(np.float32)
                + res.results[tg * EG + 1]["ytp"].astype(np.float32))
        out[tg * TC:(tg + 1) * TC, :] = part.T
    return out.reshape(B, S, D)
